# revision 29
# baseline (speedup 1.0000x reference)
"""Trainium2 Bass kernel for MultiScaleChannelTransformerBlock.

kernel(**inputs) takes the FULL inputs (as produced by setup_inputs())
and returns the FULL output [2, 128, 256, 256] float32.

Sharding: spatial over H across 8 NeuronCores (32 rows each, plus a
1-row halo on each side, host-padded).  The only cross-core
communication is an AllReduce of the per-(batch,scale) attention Gram
matrices (q/k norms come from the Gram diagonal).

v2 design notes vs the original baseline:
 - x and yhat live in bf16 (halves DMA + SBUF, full-speed PE rhs).
 - Attention epilogue is algebraic: per batch the whole
   v = wv@y, attn@v, wproj@() chain plus the +y*n1w residual collapses
   into ONE 128x128 matrix M2_b = wproj@Abd_b@wv + diag(n1w) computed
   on device after softmax; phase 2a is then a single matmul plus the
   I@x accumulate and one scalar-engine copy per chunk (no vector or
   gpsimd work at all).
 - FFN folded 3x3 dwconv runs in fp8e4 with DoubleRow pairs: taps are
   paired (dy0,dy1)x3 via row-stride APs and (dx0,dx1)+(dx2,zero) on
   the dy=2 row, so 9 bf16 matmuls become 5 DR matmuls per 128-out
   block (1.8x tensor time).  Weights are pre-scaled by S=1024 to sit
   in e4m3 normal range; 1/S is folded into gelu's scale and wpo.
 - q/k norms are read off the AllReduced Gram diagonal (the separate
   squared-norm accumulation pass is gone).
 - LN chunks are batch-paired 1024px (N=1024 bf16 matmul sums).
 - FFN/qk weights are bf16 (fast weight load); all weight DMAs issue
   at kernel start.
"""

import numpy as np
import ml_dtypes

B = 2
C = 128
CO = 32
HID = 340
W = 256
WP = W + 3            # padded z row: [pad0 | 256 data | pad | scratch]
SCALES = [1, 2, 4, 8]
NS = len(SCALES)
NU = NS * B
NTAPS = sum(r * r for r in SCALES)  # 85
EPS = 1e-5
FS = 1024.0           # fp8 weight scale

_CACHE = {}


def _build(nrows, n_cores, need_n2b, debug=False):
    import concourse.bass as bass
    import concourse.tile as tile
    from concourse import bacc, mybir

    f32 = mybir.dt.float32
    bf16 = mybir.dt.bfloat16
    fp8 = mybir.dt.float8e4
    AF = mybir.ActivationFunctionType
    OP = mybir.AluOpType
    AX = mybir.AxisListType
    DR = mybir.MatmulPerfMode.DoubleRow

    assert nrows % 16 == 0
    SLAB = nrows + 2
    NCH2 = SLAB // 2                  # batch-paired 2-row LN1 chunks
    NBAND = nrows // 16
    MH = [C, C, HID - 2 * C]          # FFN hidden blocks: 128,128,84

    nc = bacc.Bacc("TRN2", target_bir_lowering=False, debug=False,
                   num_devices=n_cores)

    def din(name, shape, dt=f32):
        return nc.dram_tensor(name, shape, dt, kind="ExternalInput").ap()

    xs = din("xs", [B, C, SLAB, W], bf16)
    wqk = din("wqk", [NTAPS, C, 2 * CO], bf16)
    bqk_d = din("bqk", [2 * CO, NS])
    wvf_d = din("wvf", [C, C], bf16)      # wv (n1w folded), [v, y] untransposed
    wpjT_d = din("wpjT", [C, C], bf16)    # wproj^T
    dn1w_d = din("dn1w", [C, C], bf16)    # diag(n1w)
    i128_d = din("i128", [C, C], bf16)    # identity
    ones_d = din("ones", [C, C], bf16)
    eye2 = din("eye2", [2 * CO, CO])
    eye64_d = din("eye64", [2 * CO, 2 * CO])
    bv_d = din("bv", [C, 1], bf16)        # wv@n1b + bv
    n1b_d = din("n1b", [C, 1])
    n2w_d = din("n2w", [C, 1])
    n2b_d = din("n2b", [C, 1])
    wf_d = din("wf", [C, 3 * 2 * 5 * 2 * C], fp8)   # [C,p,h,j,i,128]
    wpo_d = din("wpo", [C, 3, C], fp8)    # [hid-in, hid-block, ch-out] *256
    tvec = din("tvec", [1, NS])
    hmask = din("hmask", [1, 2])
    out_d = nc.dram_tensor("out", [B, C, nrows, W], f32,
                           kind="ExternalOutput").ap()

    with tile.TileContext(nc) as tc:
        with tc.tile_pool(name="wpers", bufs=1) as wp, \
             tc.tile_pool(name="xbig", bufs=1) as xp:

            def load(nm, shape, src, dt=f32):
                # weights go over the scalar queue so the x DMA owns sync
                t = wp.tile(shape, dt, name=nm, tag=nm)
                nc.scalar.dma_start(t[:], src)
                return t

            wvf_s = load("wvf_s", [C, C], wvf_d[:], dt=bf16)
            wpjT_s = load("wpjT_s", [C, C], wpjT_d[:], dt=bf16)
            dn1w_s = load("dn1w_s", [C, C], dn1w_d[:], dt=bf16)
            i128_s = load("i128_s", [C, C], i128_d[:], dt=bf16)
            ones_s = load("ones_s", [C, C], ones_d[:], dt=bf16)
            eye_s = load("eye_s", [2 * CO, CO], eye2[:])
            eye64_s = load("eye64_s", [2 * CO, 2 * CO], eye64_d[:])
            bv_s = load("bv_s", [C, 1], bv_d[:], dt=bf16)
            n1b_s = load("n1b_s", [C, 1], n1b_d[:])
            n2w_s = load("n2w_s", [C, 1], n2w_d[:])
            n2b_s = load("n2b_s", [C, 1], n2b_d[:])
            bqk_s = load("bqk_s", [2 * CO, NS], bqk_d[:])
            wf_s = load("wf_s", [C, 3 * 2 * 5 * 2 * C], wf_d[:], dt=fp8)
            wpo_s = load("wpo_s", [C, 3, C], wpo_d[:], dt=fp8)
            t_s = load("t_s", [2 * CO, NS],
                       bass.AP(tensor=tvec.tensor, offset=tvec.offset,
                               ap=[[0, 2 * CO], [1, NS]]))
            hm_s = load("hm_s", [C, 2],
                        bass.AP(tensor=hmask.tensor, offset=hmask.offset,
                                ap=[[0, C], [1, 2]]))

            accs = [wp.tile([2 * CO, B, 2 * CO], f32, name=f"accs{si}",
                            tag=f"accs{si}") for si in range(NS)]
            if n_cores > 1:
                accr = [wp.tile([2 * CO, B, 2 * CO], f32, name=f"accr{si}",
                                tag=f"accr{si}") for si in range(NS)]
            else:
                accr = accs
            M2T_s = wp.tile([C, B, C], bf16)
            Abd_s = wp.tile([C, B, C], bf16)
            d_s = wp.tile([C, B], f32)
            eps_s = wp.tile([C, 1], f32)
            nc.vector.memset(eps_s[:], EPS)

            x_s = xp.tile([C, B, SLAB, W], bf16)
            xsr = xs.rearrange("b c h w -> c b h w")
            rsplit = [(0, 9), (9, 9), (18, 8), (26, 8)]
            for r0, nr in rsplit:
                nc.sync.dma_start(x_s[:, :, r0:r0 + nr, :],
                                  bass.AP(tensor=xsr.tensor,
                                          offset=xsr.offset + r0 * W,
                                          ap=[xsr.ap[0], xsr.ap[1],
                                              [W, nr], [1, W]]))

            def flat2d(t, n):
                return bass.AP(tensor=t.tensor, offset=t.offset,
                               ap=[[t.ap[0][0], C], [1, n]])

            def layernorm_chunk(xc, write_out, lp, lps, bsplit):
                """LN over partitions.  xc: [C, B, 2, W] (bsplit=True,
                matmul per batch to respect the 512-f32 PSUM bank limit)
                or [C, 2, W].  write_out(dmu, rstd) emits the final
                normalized write (dmu * rstd [* scale])."""
                sh = [C] + list(xc.shape[1:])
                sq = lp.tile(sh, bf16, name="sq", tag="lnA")
                nc.scalar.activation(sq[:], xc, AF.Square)
                s12 = lps.tile([C, 2] + list(xc.shape[1:]), f32, tag="s12")
                if bsplit:
                    for b in range(B):
                        nc.tensor.matmul(s12[:, 0, b], ones_s[:], xc[:, b],
                                         start=True, stop=True)
                        nc.tensor.matmul(s12[:, 1, b], ones_s[:], sq[:, b],
                                         start=True, stop=True)
                else:
                    nc.tensor.matmul(s12[:, 0], ones_s[:], xc,
                                     start=True, stop=True)
                    nc.tensor.matmul(s12[:, 1], ones_s[:], sq[:],
                                     start=True, stop=True)
                mu2 = lp.tile(sh, f32, name="mu2", tag="lnB")
                nc.scalar.activation(mu2[:], s12[:, 0], AF.Square,
                                     scale=1.0 / C)
                var = lp.tile(sh, f32, name="var", tag="lnC")
                nc.vector.scalar_tensor_tensor(
                    var[:], s12[:, 1], 1.0 / C, mu2[:], OP.mult, OP.subtract)
                sig = lp.tile(sh, f32, name="sig", tag="lnA")
                nc.scalar.activation(sig[:], var[:], AF.Sqrt,
                                     bias=eps_s[:, 0:1])
                rstd = lp.tile(sh, f32, name="rstd", tag="lnB")
                nfree = int(np.prod(sh[1:]))
                nc.vector.reciprocal_approx_fast(flat2d(rstd, nfree),
                                                 flat2d(sig, nfree))
                dmu = lp.tile(sh, f32, name="dmu", tag="lnC")
                nc.vector.scalar_tensor_tensor(
                    dmu[:], s12[:, 0], -1.0 / C, xc, OP.mult, OP.add)
                write_out(dmu, rstd)

            # ==========================================================
            # Phase 1: LN1 -> yhat (bf16, raw normalized; affine folded
            # into downstream weights)
            # ==========================================================
            with tc.tile_pool(name="ybig", bufs=1) as yp:
                yhat = yp.tile([C, B, SLAB, W], bf16)

                with tc.tile_pool(name="ln1", bufs=3) as lp, \
                     tc.tile_pool(name="ln1ps", bufs=2, space="PSUM") as lps:
                    for rp in range(NCH2):
                        xc = x_s[:, :, 2 * rp:2 * rp + 2, :]
                        yc = yhat[:, :, 2 * rp:2 * rp + 2, :]

                        def wout(dmu, rstd, yc=yc):
                            nc.vector.tensor_mul(yc, dmu[:], rstd[:])
                        layernorm_chunk(xc, wout, lp, lps, bsplit=True)

                # ---- qk convs, transpose, Gram; per-scale AllReduce +
                # softmax pipelined behind the next scale's convs -------
                with tc.tile_pool(name="qkw", bufs=1) as qwp, \
                     tc.tile_pool(name="qkTp", bufs=1) as qtp, \
                     tc.tile_pool(name="qkst", bufs=4) as qsp, \
                     tc.tile_pool(name="smx", bufs=1) as sp, \
                     tc.tile_pool(name="ccd", bufs=1, space="DRAM") as dpp, \
                     tc.tile_pool(name="qkps", bufs=2, space="PSUM") as qps, \
                     tc.tile_pool(name="smxps", bufs=1, space="PSUM") as sps, \
                     tc.tile_pool(name="grps", bufs=1, space="PSUM") as gps:
                    seg = {r: (nrows // r) * (W // r) for r in SCALES}
                    segoff = {}
                    off = 0
                    for r in SCALES:
                        segoff[r] = off
                        off += seg[r]
                    nqkT = off // 128
                    qkT = qtp.tile([C, B, nqkT, 2 * CO], bf16)
                    nc.vector.memset(Abd_s[:], 0.0)
                    A_st = sp.tile([CO, NU, CO], f32)

                    for si, r in enumerate(SCALES):
                        pr, pc = nrows // r, W // r
                        t0 = sum(s * s for s in SCALES[:si])
                        wqk_s = qwp.tile([C, r * r * 2 * CO], bf16,
                                         name="wqk_s", tag="wqk_s")
                        nc.sync.dma_start(
                            wqk_s[:],
                            bass.AP(tensor=wqk.tensor,
                                    offset=wqk.offset + t0 * C * 2 * CO,
                                    ap=[[2 * CO, C], [C * 2 * CO, r * r],
                                        [1, 2 * CO]]))
                        batched = (pr * pc) < 256
                        ppc = min(max(1, 512 // pc), pr)
                        nck = (pr + ppc - 1) // ppc
                        grams = [gps.tile([2 * CO, 2 * CO], f32,
                                          name=f"gram{b}", tag=f"g{b}")
                                 for b in range(B)]
                        nmm = [0, 0]
                        st2 = None
                        for ck in range(nck):
                            q0 = ck * ppc
                            rws = min(ppc, pr - q0)
                            npx = rws * pc
                            ps = qps.tile([2 * CO, B, npx] if batched
                                          else [2 * CO, 512], f32, tag="qps")
                            for ti in range(r * r):
                                dy, dx = divmod(ti, r)
                                lhs = wqk_s[:, ti * 2 * CO:
                                            (ti + 1) * 2 * CO]
                                if batched:
                                    rhs = yhat[:, :,
                                               1 + r * q0 + dy:
                                               1 + r * (q0 + rws):r, dx::r]
                                    o = ps[:]
                                else:
                                    rhs = yhat[:, 0,
                                               1 + r * q0 + dy:
                                               1 + r * (q0 + rws):r, dx::r]
                                    o = ps[:, :npx]
                                nc.tensor.matmul(o, lhs, rhs,
                                                 start=(ti == 0),
                                                 stop=(ti == r * r - 1))
                            if not batched:
                                ps2 = qps.tile([2 * CO, 512], f32,
                                               tag="qps2")
                                for ti in range(r * r):
                                    dy, dx = divmod(ti, r)
                                    lhs = wqk_s[:, ti * 2 * CO:
                                                (ti + 1) * 2 * CO]
                                    rhs = yhat[:, 1,
                                               1 + r * q0 + dy:
                                               1 + r * (q0 + rws):r, dx::r]
                                    nc.tensor.matmul(ps2[:, :npx], lhs,
                                                     rhs,
                                                     start=(ti == 0),
                                                     stop=(ti == r * r - 1))
                            # copy PSUM -> st2 (pairing two 512px chunks per
                            # transpose), then transpose + gram when full
                            pairw = (not batched) and npx == 512 and nck > 1
                            half = ck % 2 if pairw else 0
                            if half == 0:
                                st2 = qsp.tile([2 * CO, B, 1024], bf16,
                                               tag="st2")
                                st2_base = (segoff[r] + q0 * pc) // 128
                            for b in range(B):
                                if batched:
                                    psb = ps[:, b, :]
                                elif b == 0:
                                    psb = ps[:, :npx]
                                else:
                                    psb = ps2[:, :npx]
                                nc.scalar.activation(
                                    st2[:, b, 512 * half:512 * half + npx],
                                    psb, AF.Identity,
                                    bias=bqk_s[:, si:si + 1])
                            flush = (not pairw) or half == 1 or ck == nck - 1
                            if flush:
                                filled = 512 * half + npx
                                nt = filled // 128
                                for b in range(B):
                                    nc.sync.dma_start_transpose(
                                        qkT[:, b, st2_base:st2_base + nt, :],
                                        st2[:, b, :filled])
                                    for j in range(nt):
                                        nc.tensor.matmul(
                                            grams[b][:],
                                            qkT[:, b, st2_base + j, :],
                                            qkT[:, b, st2_base + j, :],
                                            start=(nmm[b] == 0),
                                            stop=(ck == nck - 1 and
                                                  j == nt - 1),
                                            skip_group_check=True)
                                        nmm[b] += 1
                        for b in range(B):
                            nc.scalar.activation(
                                accs[si][:, b, :],
                                grams[b][:], AF.Identity, bias=0.0)

                        # ---- per-scale AllReduce of this scale's grams
                        def accflat(t):
                            return bass.AP(tensor=t.tensor, offset=t.offset,
                                           ap=[[t.ap[0][0], 2 * CO],
                                               [1, B * 2 * CO]])
                        if n_cores > 1:
                            inb = dpp.tile([2 * CO, B * 2 * CO], f32,
                                           tag=f"in{si}")
                            outb = dpp.tile([2 * CO, B * 2 * CO], f32,
                                            tag=f"out{si}")
                            nc.gpsimd.dma_start(inb[:], accflat(accs[si]))
                            nc.gpsimd.collective_compute(
                                "AllReduce", OP.add,
                                replica_groups=[list(range(n_cores))],
                                ins=[inb.opt()], outs=[outb.opt()])
                            nc.gpsimd.dma_start(accflat(accr[si]), outb[:])

                    # ---- softmax -> Abd blocks (after all scales so no
                    # collective-dependent op blocks the engine FIFOs) --
                    for si, r in enumerate(SCALES):
                        dtmp = sp.tile([2 * CO, B, 2 * CO], f32, tag="dtmp")
                        nc.vector.scalar_tensor_tensor(
                            dtmp[:], accr[si][:], 1.0,
                            bass.AP(tensor=eye64_s.tensor,
                                    offset=eye64_s.offset,
                                    ap=[[eye64_s.ap[0][0], 2 * CO],
                                        [0, B], [1, 2 * CO]]),
                            OP.bypass, OP.mult)
                        nsq = sp.tile([2 * CO, B], f32, tag="nsq")
                        nc.vector.reduce_sum(nsq[:], dtmp[:], axis=AX.X)
                        nrm = sp.tile([2 * CO, B], f32, tag="nrm")
                        nc.scalar.activation(nrm[:], nsq[:], AF.Sqrt)
                        nc.vector.tensor_scalar(nrm[:], nrm[:], 1e-12, None,
                                                OP.max)
                        rn = sp.tile([2 * CO, B], f32, tag="rn")
                        nc.vector.reciprocal(rn[:], nrm[:])
                        for b in range(B):
                            u = si * B + b
                            rq2 = sp.tile([2 * CO, 1], f32, tag="rq2")
                            nc.vector.tensor_mul(rq2[32:64, :],
                                                 rn[32:64, b:b + 1],
                                                 t_s[32:64, si:si + 1])
                            dq = sp.tile([2 * CO, CO], f32, tag="dq")
                            nc.vector.tensor_scalar(dq[32:64, :],
                                                    eye_s[32:64, :],
                                                    rq2[32:64, :], None,
                                                    OP.mult)
                            m1 = sps.tile([CO, CO], f32, tag="m1")
                            nc.tensor.matmul(m1[:], accr[si][32:64, b, 0:CO],
                                             dq[32:64, :], start=True,
                                             stop=True)
                            o1 = sp.tile([CO, CO], f32, tag="o1")
                            nc.scalar.activation(o1[:], m1[:], AF.Identity,
                                                 bias=0.0)
                            dk = sp.tile([CO, CO], f32, tag="dk")
                            nc.vector.tensor_scalar(dk[:], eye_s[0:32, :],
                                                    rn[0:32, b:b + 1], None,
                                                    OP.mult)
                            m2 = sps.tile([CO, CO], f32, tag="m2")
                            nc.tensor.matmul(m2[:], o1[:], dk[:],
                                             start=True, stop=True)
                            nc.scalar.activation(A_st[:, u, :], m2[:],
                                                 AF.Identity, bias=0.0)
                            ngm = sp.tile([CO, 1], f32, tag="ngm")
                            nc.vector.reduce_max(ngm[:], A_st[:, u, :],
                                                 axis=AX.X, negate=True)
                            Eu = sp.tile([CO, CO], f32, tag="Eu")
                            nc.scalar.activation(Eu[:], A_st[:, u, :],
                                                 AF.Exp, bias=ngm[:, 0:1])
                            ssu = sp.tile([CO, 1], f32, tag="ssu")
                            nc.vector.reduce_sum(ssu[:], Eu[:], axis=AX.X)
                            rs = sp.tile([CO, 1], f32, tag="rs")
                            nc.vector.reciprocal(rs[:], ssu[:])
                            at = sp.tile([CO, CO], bf16, tag="at")
                            nc.vector.tensor_scalar(at[:], Eu[:],
                                                    rs[:, 0:1], None,
                                                    OP.mult)
                            nc.sync.dma_start(
                                Abd_s[32 * si:32 * (si + 1), b,
                                      32 * si:32 * (si + 1)], at[:])

                # ---- fused per-batch matrix M2_b and bias d_b --------
                with tc.tile_pool(name="m2w", bufs=1) as mp, \
                     tc.tile_pool(name="m2ps", bufs=1, space="PSUM") as sps2:
                    for b in range(B):
                        tm = sps2.tile([C, C], f32, tag="tm")
                        nc.tensor.matmul(tm[:], Abd_s[:, b, :], wpjT_s[:],
                                         start=True, stop=True)
                        tm_sb = mp.tile([C, C], bf16, tag="tm_sb")
                        nc.scalar.activation(tm_sb[:], tm[:], AF.Identity,
                                             bias=0.0)
                        m2ps = sps2.tile([C, C], f32, tag="m2ps")
                        nc.tensor.matmul(m2ps[:], wvf_s[:], tm_sb[:],
                                         start=True, stop=False)
                        nc.tensor.matmul(m2ps[:], dn1w_s[:], i128_s[:],
                                         start=False, stop=True)
                        nc.scalar.activation(M2T_s[:, b, :], m2ps[:],
                                             AF.Identity, bias=0.0)
                        dps = sps2.tile([C, 1], f32, tag="dps")
                        nc.tensor.matmul(dps[:], tm_sb[:], bv_s[:],
                                         start=True, stop=True)
                        nc.scalar.activation(d_s[:, b:b + 1], dps[:],
                                             AF.Identity,
                                             bias=n1b_s[:, 0:1])

                # ---- phase 2a: x_mid = x + M2@yhat + d ---------------
                with tc.tile_pool(name="p2aps", bufs=3, space="PSUM") as aps:
                    for ch in range(B * NCH2):
                        b, rp = divmod(ch, NCH2)
                        yc = yhat[:, b, 2 * rp:2 * rp + 2, :]
                        xc = x_s[:, b, 2 * rp:2 * rp + 2, :]
                        pj = aps.tile([C, 2, W], f32, tag="pj")
                        nc.tensor.matmul(pj[:], M2T_s[:, b, :], yc,
                                         start=True, stop=False)
                        nc.tensor.matmul(pj[:], i128_s[:], xc,
                                         start=False, stop=True)
                        nc.scalar.activation(xc, pj[:], AF.Identity,
                                             bias=d_s[:, b:b + 1])
            # yhat freed here

            # ==========================================================
            # Phase 2b: LN2 + folded FFN (fp8 DoubleRow), band by band
            # ==========================================================
            with tc.tile_pool(name="zp", bufs=1) as zp, \
                 tc.tile_pool(name="ln2", bufs=3) as lp2, \
                 tc.tile_pool(name="ln2ps", bufs=1, space="PSUM") as lps2, \
                 tc.tile_pool(name="fps", bufs=2, space="PSUM") as fps, \
                 tc.tile_pool(name="ops", bufs=2, space="PSUM") as ops_, \
                 tc.tile_pool(name="gp", bufs=2) as gp, \
                 tc.tile_pool(name="outp", bufs=3) as outp:

                zts = []
                for bd in range(NBAND):
                    zr0 = 16 * bd
                    zt = zp.tile([C, B, 18, WP], fp8, tag=f"zt{bd % 2}")
                    zts.append(zt)
                    for ch in range(2 * 9):
                        b, rp = divmod(ch, 9)
                        xc = x_s[:, b, zr0 + 2 * rp:zr0 + 2 * rp + 2, :]

                        def wout(dmu, rstd, zt=zt, b=b, rp=rp):
                            nc.vector.scalar_tensor_tensor(
                                zt[:, b, 2 * rp:2 * rp + 2, 1:W + 1],
                                dmu[:], n2w_s[:, 0:1], rstd[:],
                                OP.mult, OP.mult)
                        layernorm_chunk(xc, wout, lp2, lps2, bsplit=False)
                    if need_n2b:
                        nc.scalar.activation(zt[:, :, :, 1:W + 1],
                                             zt[:, :, :, 1:W + 1],
                                             AF.Identity,
                                             bias=n2b_s[:, 0:1])
                    # zero pad columns (left, right, scratch)
                    nc.vector.memset(zt[:, :, :, 0:1], 0.0)
                    nc.vector.memset(zt[:, :, :, W + 1:W + 3], 0.0)
                    # zero halo rows at image boundary
                    if bd == 0:
                        nc.vector.tensor_scalar(
                            zt[:, :, 0, 1:W + 1], zt[:, :, 0, 1:W + 1],
                            hm_s[:, 0:1], None, OP.mult)
                    if bd == NBAND - 1:
                        nc.vector.tensor_scalar(
                            zt[:, :, 17, 1:W + 1], zt[:, :, 17, 1:W + 1],
                            hm_s[:, 1:2], None, OP.mult)

                def dr_rhs(zt, b, rp, j):
                    """DoubleRow moving operand for tap-pair j of the
                    2-row chunk rp (batch b)."""
                    base = (zt.offset + b * 18 * WP + 2 * rp * WP)
                    pstride = zt.ap[0][0]
                    if j < 3:
                        return bass.AP(tensor=zt.tensor, offset=base + j,
                                       ap=[[pstride, C], [WP, 2],
                                           [WP, 2], [1, W]])
                    col = 0 if j == 3 else 2
                    return bass.AP(tensor=zt.tensor,
                                   offset=base + 2 * WP + col,
                                   ap=[[pstride, C], [1, 2],
                                       [WP, 2], [1, W]])

                def dr_lhs(p, h, j, mh):
                    off = (((p * 2 + h) * 5 + j) * 2) * C
                    return bass.AP(tensor=wf_s.tensor,
                                   offset=wf_s.offset + off,
                                   ap=[[wf_s.ap[0][0], C], [C, 2], [1, mh]])

                for bd in range(NBAND):
                    zr0 = 16 * bd
                    zt = zts[bd]
                    for ch in range(16):
                        b, rp = divmod(ch, 8)
                        ops = ops_.tile([C, 2, W], f32, tag="ops")
                        g01 = gp.tile([C, 2, 2, W], fp8, tag="g01")
                        g2 = gp.tile([C, 2, W], fp8, tag="g2")
                        for p in range(3):
                            mh = MH[p]
                            f1 = fps.tile([C, 2, W], f32, tag="f1")
                            f2 = fps.tile([C, 2, W], f32, tag="f2")
                            for j in range(5):
                                rhs = dr_rhs(zt, b, rp, j)
                                nc.tensor.matmul(
                                    f1[:mh], dr_lhs(p, 0, j, mh),
                                    rhs, start=(j == 0), stop=(j == 4),
                                    perf_mode=DR)
                                nc.tensor.matmul(
                                    f2[:mh], dr_lhs(p, 1, j, mh),
                                    rhs, start=(j == 0), stop=(j == 4),
                                    perf_mode=DR)
                            g1 = gp.tile([C, 2, W], f32, tag="g1")
                            nc.scalar.activation(g1[:mh], f1[:mh], AF.Gelu,
                                                 scale=1.0 / FS)
                            # g8 = 16 * gelu(f1) * f2  (fp8)
                            gdst = g01[:, p] if p < 2 else g2[:mh]
                            nc.vector.scalar_tensor_tensor(
                                gdst, g1[:mh], 16.0 / FS, f2[:mh],
                                OP.mult, OP.mult)
                        # out = (wpo*256) @ g8 = 4096 * out_true
                        nc.tensor.matmul(ops[:], wpo_s[:, 0:2, :], g01[:],
                                         start=True, stop=False,
                                         perf_mode=DR)
                        nc.tensor.matmul(ops[:], wpo_s[:MH[2], 2, :],
                                         g2[:MH[2]],
                                         start=False, stop=True)
                        o_sb = outp.tile([C, 2, W], f32, tag="o_sb")
                        nc.vector.scalar_tensor_tensor(
                            o_sb[:], ops[:], 1.0 / 4096.0,
                            x_s[:, b, zr0 + 1 + 2 * rp:zr0 + 3 + 2 * rp, :],
                            OP.mult, OP.add)
                        gr = 16 * bd + 2 * rp
                        nc.gpsimd.dma_start(out_d[b, :, gr:gr + 2, :],
                                            o_sb[:])

    nc.compile()
    return nc


# ---------------------------------------------------------------------------
# host side
# ---------------------------------------------------------------------------

def _prep_inputs(inputs, nrows, n_cores):
    H = nrows * n_cores
    x = np.asarray(inputs["x"], np.float32)
    n1w = np.asarray(inputs["n1w"], np.float32)
    n1b = np.asarray(inputs["n1b"], np.float32)
    n2w = np.asarray(inputs["n2w"], np.float32)
    n2b = np.asarray(inputs["n2b"], np.float32)

    wqk_taps = np.zeros((NTAPS, C, 2 * CO), np.float32)
    bqk = np.zeros((2 * CO, NS), np.float32)
    ti = 0
    for si, r in enumerate(SCALES):
        wqk = np.asarray(inputs[f"wqk{si}"], np.float32)  # [64,128,r,r]
        wqkf = wqk * n1w[None, :, None, None]
        bqk[:, si] = np.einsum("ocyx,c->o", wqk, n1b)
        for dy in range(r):
            for dx in range(r):
                wqk_taps[ti] = wqkf[:, :, dy, dx].T
                ti += 1

    wv_cat = np.concatenate([np.asarray(inputs[f"wv{i}"], np.float32)[:, :, 0, 0]
                             for i in range(NS)], axis=0)      # [128,128]
    bv_cat = np.concatenate([np.asarray(inputs[f"bv{i}"], np.float32)
                             for i in range(NS)])
    bv_all = (wv_cat @ n1b + bv_cat).astype(np.float32)
    wv_f = (wv_cat * n1w[None, :])                             # [v, y]

    wpjT = np.asarray(inputs["wproj"], np.float32)[:, :, 0, 0].T.copy()

    wpi = np.asarray(inputs["wpi"], np.float32)[:, :, 0, 0]    # [680,128]
    wdw = np.asarray(inputs["wdw"], np.float32)[:, 0]          # [680,3,3]
    wf_full = np.zeros((9, C, 2 * HID), np.float32)
    for ti in range(9):
        dy, dx = divmod(ti, 3)
        wf_full[ti] = (wpi * wdw[:, dy, dx][:, None]).T        # [128,680]
    assert np.abs(wf_full).max() * FS < 230.0, "fp8 weight scale overflow"
    # DoubleRow pair table: (j, member) -> tap index (dy*3+dx)
    PAIRS = [(0, 3), (1, 4), (2, 5), (6, 7), (8, None)]
    wf8 = np.zeros((C, 3, 2, 5, 2, C), np.float32)
    for p in range(3):
        mh = min(C, HID - C * p)
        for h in range(2):
            c0 = h * HID + p * C
            for j, pair in enumerate(PAIRS):
                for i, tap in enumerate(pair):
                    if tap is None:
                        continue
                    wf8[:, p, h, j, i, :mh] = FS * wf_full[tap][:, c0:c0 + mh]
    wf8 = wf8.reshape(C, -1).astype(ml_dtypes.float8_e4m3)

    wpo = np.asarray(inputs["wpo"], np.float32)[:, :, 0, 0]    # [128,340]
    wpo_p = np.zeros((C, 3, C), np.float32)
    for p in range(3):
        mh = min(C, HID - C * p)
        wpo_p[:mh, p, :] = wpo[:, C * p:C * p + mh].T * 256.0
    assert np.abs(wpo_p).max() < 230.0, "fp8 wpo scale overflow"

    tv = np.array([[float(np.asarray(inputs[f"t{i}"]).reshape(-1)[0])
                    for i in range(NS)]], np.float32)
    eye2 = np.concatenate([np.eye(CO, dtype=np.float32)] * 2, axis=0)

    bf = ml_dtypes.bfloat16
    shared = {
        "wqk": wqk_taps.astype(bf), "bqk": bqk,
        "wvf": wv_f.astype(bf), "wpjT": wpjT.astype(bf),
        "dn1w": np.diag(n1w).astype(bf),
        "i128": np.eye(C, dtype=np.float32).astype(bf),
        "ones": np.ones((C, C), np.float32).astype(bf),
        "eye2": eye2, "eye64": np.eye(2 * CO, dtype=np.float32),
        "bv": bv_all.reshape(C, 1).astype(bf),
        "n1b": n1b.reshape(C, 1), "n2w": n2w.reshape(C, 1),
        "n2b": n2b.reshape(C, 1),
        "wf": wf8, "wpo": wpo_p.astype(ml_dtypes.float8_e4m3), "tvec": tv,
    }
    need_n2b = bool(np.any(n2b != 0.0))

    in_maps = []
    for i in range(n_cores):
        r0 = nrows * i
        slab = np.zeros((B, C, nrows + 2, W), np.float32)
        lo, hi = r0 - 1, r0 + nrows + 1
        slo, shi = max(lo, 0), min(hi, H)
        slab[:, :, slo - lo:shi - lo, :] = x[:, :, slo:shi, :]
        m = {"xs": slab.astype(bf),
             "hmask": np.array([[1.0 if i > 0 else 0.0,
                                 1.0 if i < n_cores - 1 else 0.0]],
                               np.float32)}
        m.update(shared)
        in_maps.append(m)
    return in_maps, need_n2b


def _run(nrows, n_cores, in_maps, need_n2b, trace=False):
    from concourse.bass_utils import run_bass_kernel_spmd
    key = (nrows, n_cores, need_n2b)
    if key not in _CACHE:
        _CACHE[key] = _build(nrows, n_cores, need_n2b)
    nc = _CACHE[key]
    return run_bass_kernel_spmd(nc, in_maps, core_ids=list(range(n_cores)),
                                trace=trace)


def run_sharded(inputs, nrows=32, n_cores=8, trace=False):
    in_maps, need_n2b = _prep_inputs(inputs, nrows, n_cores)
    res = _run(nrows, n_cores, in_maps, need_n2b, trace=trace)
    H = nrows * n_cores
    out = np.zeros((B, C, H, W), np.float32)
    for i in range(n_cores):
        out[:, :, nrows * i:nrows * (i + 1), :] = res.results[i]["out"]
    return out, res


def kernel(**inputs):
    out, _ = run_sharded(inputs, nrows=32, n_cores=8)
    return out


# revision 49
# speedup vs baseline: 1.1641x; 1.1641x over previous
"""Trainium2 Bass kernel for MultiScaleChannelTransformerBlock.

kernel(**inputs) takes the FULL inputs (as produced by setup_inputs())
and returns the FULL output [2, 128, 256, 256] float32.

Sharding: spatial over H across 8 NeuronCores (32 rows each, plus a
1-row halo on each side, host-padded).  The only cross-core
communication is an AllReduce of the per-(batch,scale) attention Gram
matrices (q/k norms come from the Gram diagonal).

v2 design notes vs the original baseline:
 - x and yhat live in bf16 (halves DMA + SBUF, full-speed PE rhs).
 - Attention epilogue is algebraic: per batch the whole
   v = wv@y, attn@v, wproj@() chain plus the +y*n1w residual collapses
   into ONE 128x128 matrix M2_b = wproj@Abd_b@wv + diag(n1w) computed
   on device after softmax; phase 2a is then a single matmul plus the
   I@x accumulate and one scalar-engine copy per chunk (no vector or
   gpsimd work at all).
 - FFN folded 3x3 dwconv runs in fp8e4 with DoubleRow pairs: taps are
   paired (dy0,dy1)x3 via row-stride APs and (dx0,dx1)+(dx2,zero) on
   the dy=2 row, so 9 bf16 matmuls become 5 DR matmuls per 128-out
   block (1.8x tensor time).  Weights are pre-scaled by S=1024 to sit
   in e4m3 normal range; 1/S is folded into gelu's scale and wpo.
 - q/k norms are read off the AllReduced Gram diagonal (the separate
   squared-norm accumulation pass is gone).
 - LN chunks are batch-paired 1024px (N=1024 bf16 matmul sums).
 - FFN/qk weights are bf16 (fast weight load); all weight DMAs issue
   at kernel start.
"""

import numpy as np
import ml_dtypes

B = 2
C = 128
CO = 32
HID = 340
W = 256
WP = W + 3            # padded z row: [pad0 | 256 data | pad | scratch]
SCALES = [1, 2, 4, 8]
NS = len(SCALES)
NU = NS * B
NTAPS = sum(r * r for r in SCALES)  # 85
EPS = 1e-5
FS = 1024.0           # fp8 weight scale

_CACHE = {}


def _build(nrows, n_cores, need_n2b, nonce="0", debug=False):
    import concourse.bass as bass
    import concourse.tile as tile
    from concourse import bacc, mybir

    f32 = mybir.dt.float32
    bf16 = mybir.dt.bfloat16
    fp8 = mybir.dt.float8e4
    AF = mybir.ActivationFunctionType
    OP = mybir.AluOpType
    AX = mybir.AxisListType
    DR = mybir.MatmulPerfMode.DoubleRow

    assert nrows % 16 == 0
    SLAB = nrows + 2
    NCH2 = SLAB // 2                  # batch-paired 2-row LN1 chunks
    NBAND = nrows // 16
    MH = [C, C, HID - 2 * C]          # FFN hidden blocks: 128,128,84

    nc = bacc.Bacc("TRN2", target_bir_lowering=False, debug=False,
                   num_devices=n_cores)

    def din(name, shape, dt=f32):
        return nc.dram_tensor(name, shape, dt, kind="ExternalInput").ap()

    xs = din("xs", [B, C, SLAB, W], bf16)
    wqk = din("wqk", [NTAPS, C, 2 * CO], bf16)
    bqk_d = din("bqk", [2 * CO, NS])
    wvf_d = din("wvf", [C, C], bf16)      # wv (n1w folded), [v, y] untransposed
    wpjT_d = din("wpjT", [C, C], bf16)    # wproj^T
    dn1w_d = din("dn1w", [C, C], bf16)    # diag(n1w)
    i128_d = din("i128", [C, C], bf16)    # identity
    ones_d = din("ones", [C, C], bf16)
    eye2 = din("eye2", [2 * CO, CO])
    eye64_d = din("eye64", [2 * CO, 2 * CO])
    bv_d = din("bv", [C, 1], bf16)        # wv@n1b + bv
    n1b_d = din("n1b", [C, 1])
    n2w_d = din("n2w", [C, 1])
    n2b_d = din("n2b", [C, 1])
    wf_d = din("wf", [C, 3 * 2 * 5 * 2 * C], fp8)   # [C,p,h,j,i,128]
    wpo_d = din("wpo", [C, 3, C], fp8)    # [hid-in, hid-block, ch-out] *256
    tvec = din("tvec", [1, NS])
    hmask = din("hmask", [1, 2])

    out_d = nc.dram_tensor("out", [B, C, nrows, W], f32,
                           kind="ExternalOutput").ap()
    if debug:
        dbg_y = nc.dram_tensor("dbg_y", [C, B, SLAB, W], bf16,
                               kind="ExternalOutput").ap()
        dbg_xm = nc.dram_tensor("dbg_xm", [C, B, SLAB, W], bf16,
                                kind="ExternalOutput").ap()
        dbg_acc = nc.dram_tensor("dbg_acc", [2 * CO, NS, B * 2 * CO], f32,
                                 kind="ExternalOutput").ap()
        dbg_m2 = nc.dram_tensor("dbg_m2", [C, B, C], bf16,
                                kind="ExternalOutput").ap()
        dbg_xin = nc.dram_tensor("dbg_xin", [C, B, SLAB, W], bf16,
                                 kind="ExternalOutput").ap()

    with tile.TileContext(nc) as tc:
        with tc.tile_pool(name="wpers", bufs=1) as wp, \
             tc.tile_pool(name="xbig", bufs=1) as xp:

            def load(nm, shape, src, dt=f32):
                t = wp.tile(shape, dt, name=nm, tag=nm)
                nc.sync.dma_start(t[:], src)
                return t

            wvf_s = load("wvf_s", [C, C], wvf_d[:], dt=bf16)
            wpjT_s = load("wpjT_s", [C, C], wpjT_d[:], dt=bf16)
            dn1w_s = load("dn1w_s", [C, C], dn1w_d[:], dt=bf16)
            i128_s = load("i128_s", [C, C], i128_d[:], dt=bf16)
            ones_s = load("ones_s", [C, C], ones_d[:], dt=bf16)
            eye_s = load("eye_s", [2 * CO, CO], eye2[:])
            eye64_s = load("eye64_s", [2 * CO, 2 * CO], eye64_d[:])
            bv_s = load("bv_s", [C, 1], bv_d[:], dt=bf16)
            n1b_s = load("n1b_s", [C, 1], n1b_d[:])
            n2w_s = load("n2w_s", [C, 1], n2w_d[:])
            n2b_s = load("n2b_s", [C, 1], n2b_d[:])
            bqk_s = load("bqk_s", [2 * CO, NS], bqk_d[:])
            wf_s = load("wf_s", [C, 3 * 2 * 5 * 2 * C], wf_d[:], dt=fp8)
            wpo_s = load("wpo_s", [C, 3, C], wpo_d[:], dt=fp8)
            t_s = load("t_s", [2 * CO, NS],
                       bass.AP(tensor=tvec.tensor, offset=tvec.offset,
                               ap=[[0, 2 * CO], [1, NS]]))
            hm_s = load("hm_s", [C, 2],
                        bass.AP(tensor=hmask.tensor, offset=hmask.offset,
                                ap=[[0, C], [1, 2]]))


            # structural nonce: a unique number of no-op memsets on the
            # (otherwise idle) gpsimd queue forces a distinct program /
            # NEFF per call, so every execution is a first run.
            scr = wp.tile([C, 2], f32, name="scr", tag="scr")
            for _ in range(3 + int(nonce, 16) % 499):
                nc.gpsimd.memset(scr[:], 0.0)

            accs = [wp.tile([2 * CO, B, 2 * CO], f32, name=f"accs{si}",
                            tag=f"accs{si}") for si in range(NS)]
            if n_cores > 1:
                accr = [wp.tile([2 * CO, B, 2 * CO], f32, name=f"accr{si}",
                                tag=f"accr{si}") for si in range(NS)]
            else:
                accr = accs
            M2T_s = wp.tile([C, B, C], bf16)
            Abd_s = wp.tile([C, B, C], bf16)
            d_s = wp.tile([C, B], f32)
            eps_s = wp.tile([C, 1], f32)
            nc.vector.memset(eps_s[:], EPS)

            # Re-executing a previously-loaded NEFF returns corrupted
            # results on this stack (device-persistent state).  Vary the
            # x-DMA row split by the per-call nonce so every build is a
            # structurally unique program — each execution is a first run.
            ksplit = 4 + (int(nonce, 16) % 24)
            x_s = xp.tile([C, B, SLAB, W], bf16)
            xsr = xs.rearrange("b c h w -> c b h w")
            for r0, nr in [(0, ksplit), (ksplit, SLAB - ksplit)]:
                nc.sync.dma_start(
                    x_s[:, :, r0:r0 + nr, :],
                    bass.AP(tensor=xsr.tensor, offset=xsr.offset + r0 * W,
                            ap=[xsr.ap[0], xsr.ap[1], [W, nr], [1, W]]))

            def flat2d(t, n):
                return bass.AP(tensor=t.tensor, offset=t.offset,
                               ap=[[t.ap[0][0], C], [1, n]])

            def layernorm_chunk(xc, write_out, lp, lps, bsplit):
                """LN over partitions.  xc: [C, B, 2, W] (bsplit=True,
                matmul per batch to respect the 512-f32 PSUM bank limit)
                or [C, 2, W].  write_out(dmu, rstd) emits the final
                normalized write (dmu * rstd [* scale])."""
                sh = [C] + list(xc.shape[1:])
                sq = lp.tile(sh, bf16, name="sq", tag="lnA")
                nc.scalar.activation(sq[:], xc, AF.Square)
                s12 = lps.tile([C, 2] + list(xc.shape[1:]), f32, tag="s12")
                if bsplit:
                    for b in range(B):
                        nc.tensor.matmul(s12[:, 0, b], ones_s[:], xc[:, b],
                                         start=True, stop=True)
                        nc.tensor.matmul(s12[:, 1, b], ones_s[:], sq[:, b],
                                         start=True, stop=True)
                else:
                    nc.tensor.matmul(s12[:, 0], ones_s[:], xc,
                                     start=True, stop=True)
                    nc.tensor.matmul(s12[:, 1], ones_s[:], sq[:],
                                     start=True, stop=True)
                mu2 = lp.tile(sh, f32, name="mu2", tag="lnB")
                nc.scalar.activation(mu2[:], s12[:, 0], AF.Square,
                                     scale=1.0 / C)
                var = lp.tile(sh, f32, name="var", tag="lnC")
                nc.vector.scalar_tensor_tensor(
                    var[:], s12[:, 1], 1.0 / C, mu2[:], OP.mult, OP.subtract)
                sig = lp.tile(sh, f32, name="sig", tag="lnA")
                nc.scalar.activation(sig[:], var[:], AF.Sqrt,
                                     bias=eps_s[:, 0:1])
                rstd = lp.tile(sh, f32, name="rstd", tag="lnB")
                nfree = int(np.prod(sh[1:]))
                nc.vector.reciprocal_approx_fast(flat2d(rstd, nfree),
                                                 flat2d(sig, nfree))
                dmu = lp.tile(sh, f32, name="dmu", tag="lnC")
                nc.vector.scalar_tensor_tensor(
                    dmu[:], s12[:, 0], -1.0 / C, xc, OP.mult, OP.add)
                write_out(dmu, rstd)

            # ==========================================================
            # Phase 1: LN1 -> yhat (bf16, raw normalized; affine folded
            # into downstream weights)
            # ==========================================================
            with tc.tile_pool(name="ybig", bufs=1) as yp:
                yhat = yp.tile([C, B, SLAB, W], bf16)

                with tc.tile_pool(name="ln1", bufs=3) as lp, \
                     tc.tile_pool(name="ln1ps", bufs=2, space="PSUM") as lps:
                    for rp in range(NCH2):
                        xc = x_s[:, :, 2 * rp:2 * rp + 2, :]
                        yc = yhat[:, :, 2 * rp:2 * rp + 2, :]

                        def wout(dmu, rstd, yc=yc):
                            nc.vector.tensor_mul(yc, dmu[:], rstd[:])
                        layernorm_chunk(xc, wout, lp, lps, bsplit=True)

                # ---- qk convs, transpose, Gram; per-scale AllReduce +
                # softmax pipelined behind the next scale's convs -------
                with tc.tile_pool(name="qkw", bufs=1) as qwp, \
                     tc.tile_pool(name="qkTp", bufs=1) as qtp, \
                     tc.tile_pool(name="qkst", bufs=4) as qsp, \
                     tc.tile_pool(name="smx", bufs=1) as sp, \
                     tc.tile_pool(name="ccd", bufs=1, space="DRAM") as dpp, \
                     tc.tile_pool(name="qkps", bufs=2, space="PSUM") as qps, \
                     tc.tile_pool(name="smxps", bufs=1, space="PSUM") as sps, \
                     tc.tile_pool(name="grps", bufs=1, space="PSUM") as gps:
                    seg = {r: (nrows // r) * (W // r) for r in SCALES}
                    segoff = {}
                    off = 0
                    for r in SCALES:
                        segoff[r] = off
                        off += seg[r]
                    nqkT = off // 128
                    qkT = qtp.tile([C, B, nqkT, 2 * CO], bf16)
                    nc.vector.memset(Abd_s[:], 0.0)
                    A_st = sp.tile([CO, NU, CO], f32)

                    def accflat(t):
                        return bass.AP(tensor=t.tensor, offset=t.offset,
                                       ap=[[t.ap[0][0], 2 * CO],
                                           [1, B * 2 * CO]])
                    if n_cores > 1:
                        inb_all = dpp.tile([2 * CO, NU * 2 * CO], f32,
                                           tag="inb")
                        outb_all = dpp.tile([2 * CO, NU * 2 * CO], f32,
                                            tag="outb")

                    for si, r in enumerate(SCALES):
                        pr, pc = nrows // r, W // r
                        t0 = sum(s * s for s in SCALES[:si])
                        wqk_s = qwp.tile([C, r * r * 2 * CO], bf16,
                                         name="wqk_s", tag="wqk_s")
                        nc.sync.dma_start(
                            wqk_s[:],
                            bass.AP(tensor=wqk.tensor,
                                    offset=wqk.offset + t0 * C * 2 * CO,
                                    ap=[[2 * CO, C], [C * 2 * CO, r * r],
                                        [1, 2 * CO]]))
                        batched = (pr * pc) < 256
                        ppc = min(max(1, 512 // pc), pr)
                        nck = (pr + ppc - 1) // ppc
                        grams = [gps.tile([2 * CO, 2 * CO], f32,
                                          name=f"gram{b}", tag=f"g{b}")
                                 for b in range(B)]
                        nmm = [0, 0]
                        st2 = None
                        for ck in range(nck):
                            q0 = ck * ppc
                            rws = min(ppc, pr - q0)
                            npx = rws * pc
                            ps = qps.tile([2 * CO, B, npx] if batched
                                          else [2 * CO, 512], f32, tag="qps")
                            for ti in range(r * r):
                                dy, dx = divmod(ti, r)
                                lhs = wqk_s[:, ti * 2 * CO:
                                            (ti + 1) * 2 * CO]
                                if batched:
                                    rhs = yhat[:, :,
                                               1 + r * q0 + dy:
                                               1 + r * (q0 + rws):r, dx::r]
                                    o = ps[:]
                                else:
                                    rhs = yhat[:, 0,
                                               1 + r * q0 + dy:
                                               1 + r * (q0 + rws):r, dx::r]
                                    o = ps[:, :npx]
                                nc.tensor.matmul(o, lhs, rhs,
                                                 start=(ti == 0),
                                                 stop=(ti == r * r - 1))
                            if not batched:
                                ps2 = qps.tile([2 * CO, 512], f32,
                                               tag="qps2")
                                for ti in range(r * r):
                                    dy, dx = divmod(ti, r)
                                    lhs = wqk_s[:, ti * 2 * CO:
                                                (ti + 1) * 2 * CO]
                                    rhs = yhat[:, 1,
                                               1 + r * q0 + dy:
                                               1 + r * (q0 + rws):r, dx::r]
                                    nc.tensor.matmul(ps2[:, :npx], lhs,
                                                     rhs,
                                                     start=(ti == 0),
                                                     stop=(ti == r * r - 1))
                            # copy PSUM -> st2 (pairing two 512px chunks per
                            # transpose), then transpose + gram when full
                            pairw = (not batched) and npx == 512 and nck > 1
                            half = ck % 2 if pairw else 0
                            if half == 0:
                                st2 = qsp.tile([2 * CO, B, 1024], bf16,
                                               tag="st2")
                                st2_base = (segoff[r] + q0 * pc) // 128
                            for b in range(B):
                                if batched:
                                    psb = ps[:, b, :]
                                elif b == 0:
                                    psb = ps[:, :npx]
                                else:
                                    psb = ps2[:, :npx]
                                nc.scalar.activation(
                                    st2[:, b, 512 * half:512 * half + npx],
                                    psb, AF.Identity,
                                    bias=bqk_s[:, si:si + 1])
                            flush = (not pairw) or half == 1 or ck == nck - 1
                            if flush:
                                filled = 512 * half + npx
                                nt = filled // 128
                                for b in range(B):
                                    eng = nc.sync if b == 0 else nc.scalar
                                    eng.dma_start_transpose(
                                        qkT[:, b, st2_base:st2_base + nt, :],
                                        st2[:, b, :filled])
                                    for j in range(nt):
                                        nc.tensor.matmul(
                                            grams[b][:],
                                            qkT[:, b, st2_base + j, :],
                                            qkT[:, b, st2_base + j, :],
                                            start=(nmm[b] == 0),
                                            stop=(ck == nck - 1 and
                                                  j == nt - 1),
                                            skip_group_check=True)
                                        nmm[b] += 1
                        for b in range(B):
                            nc.scalar.activation(
                                accs[si][:, b, :],
                                grams[b][:], AF.Identity, bias=0.0)

                        # ---- stage this scale's grams into the collective
                        # input buffer (the single AllReduce runs after the
                        # last scale; staging DMAs pipeline per scale)
                        if n_cores > 1:
                            nc.scalar.dma_start(
                                inb_all[:, si * B * 2 * CO:
                                        (si + 1) * B * 2 * CO],
                                accflat(accs[si]))

                    if n_cores > 1:
                        nc.gpsimd.collective_compute(
                            "AllReduce", OP.add,
                            replica_groups=[list(range(n_cores))],
                            ins=[inb_all.opt()], outs=[outb_all.opt()])
                        for si in range(NS):
                            nc.scalar.dma_start(
                                accflat(accr[si]),
                                outb_all[:, si * B * 2 * CO:
                                         (si + 1) * B * 2 * CO])

                    # ---- softmax -> Abd blocks (after all scales so no
                    # collective-dependent op blocks the engine FIFOs) --
                    for si, r in enumerate(SCALES):
                        dtmp = sp.tile([2 * CO, B, 2 * CO], f32, tag="dtmp")
                        nc.vector.scalar_tensor_tensor(
                            dtmp[:], accr[si][:], 1.0,
                            bass.AP(tensor=eye64_s.tensor,
                                    offset=eye64_s.offset,
                                    ap=[[eye64_s.ap[0][0], 2 * CO],
                                        [0, B], [1, 2 * CO]]),
                            OP.bypass, OP.mult)
                        nsq = sp.tile([2 * CO, B], f32, tag="nsq")
                        nc.vector.reduce_sum(nsq[:], dtmp[:], axis=AX.X)
                        nrm = sp.tile([2 * CO, B], f32, tag="nrm")
                        nc.scalar.activation(nrm[:], nsq[:], AF.Sqrt)
                        nc.vector.tensor_scalar(nrm[:], nrm[:], 1e-12, None,
                                                OP.max)
                        rn = sp.tile([2 * CO, B], f32, tag="rn")
                        nc.vector.reciprocal(rn[:], nrm[:])
                        for b in range(B):
                            u = si * B + b
                            rq2 = sp.tile([2 * CO, 1], f32, tag="rq2")
                            nc.vector.tensor_mul(rq2[32:64, :],
                                                 rn[32:64, b:b + 1],
                                                 t_s[32:64, si:si + 1])
                            dq = sp.tile([2 * CO, CO], f32, tag="dq")
                            nc.vector.tensor_scalar(dq[32:64, :],
                                                    eye_s[32:64, :],
                                                    rq2[32:64, :], None,
                                                    OP.mult)
                            m1 = sps.tile([CO, CO], f32, tag="m1")
                            nc.tensor.matmul(m1[:], accr[si][32:64, b, 0:CO],
                                             dq[32:64, :], start=True,
                                             stop=True)
                            o1 = sp.tile([CO, CO], f32, tag="o1")
                            nc.scalar.activation(o1[:], m1[:], AF.Identity,
                                                 bias=0.0)
                            dk = sp.tile([CO, CO], f32, tag="dk")
                            nc.vector.tensor_scalar(dk[:], eye_s[0:32, :],
                                                    rn[0:32, b:b + 1], None,
                                                    OP.mult)
                            m2 = sps.tile([CO, CO], f32, tag="m2")
                            nc.tensor.matmul(m2[:], o1[:], dk[:],
                                             start=True, stop=True)
                            nc.scalar.activation(A_st[:, u, :], m2[:],
                                                 AF.Identity, bias=0.0)
                            ngm = sp.tile([CO, 1], f32, tag="ngm")
                            nc.vector.reduce_max(ngm[:], A_st[:, u, :],
                                                 axis=AX.X, negate=True)
                            Eu = sp.tile([CO, CO], f32, tag="Eu")
                            nc.scalar.activation(Eu[:], A_st[:, u, :],
                                                 AF.Exp, bias=ngm[:, 0:1])
                            ssu = sp.tile([CO, 1], f32, tag="ssu")
                            nc.vector.reduce_sum(ssu[:], Eu[:], axis=AX.X)
                            rs = sp.tile([CO, 1], f32, tag="rs")
                            nc.vector.reciprocal(rs[:], ssu[:])
                            at = sp.tile([CO, CO], bf16, tag="at")
                            nc.vector.tensor_scalar(at[:], Eu[:],
                                                    rs[:, 0:1], None,
                                                    OP.mult)
                            nc.sync.dma_start(
                                Abd_s[32 * si:32 * (si + 1), b,
                                      32 * si:32 * (si + 1)], at[:])

                # ---- fused per-batch matrix M2_b and bias d_b --------
                with tc.tile_pool(name="m2w", bufs=1) as mp, \
                     tc.tile_pool(name="m2ps", bufs=1, space="PSUM") as sps2:
                    for b in range(B):
                        tm = sps2.tile([C, C], f32, tag="tm")
                        nc.tensor.matmul(tm[:], Abd_s[:, b, :], wpjT_s[:],
                                         start=True, stop=True)
                        tm_sb = mp.tile([C, C], bf16, tag="tm_sb")
                        nc.scalar.activation(tm_sb[:], tm[:], AF.Identity,
                                             bias=0.0)
                        m2ps = sps2.tile([C, C], f32, tag="m2ps")
                        nc.tensor.matmul(m2ps[:], wvf_s[:], tm_sb[:],
                                         start=True, stop=False)
                        nc.tensor.matmul(m2ps[:], dn1w_s[:], i128_s[:],
                                         start=False, stop=True)
                        nc.scalar.activation(M2T_s[:, b, :], m2ps[:],
                                             AF.Identity, bias=0.0)
                        dps = sps2.tile([C, 1], f32, tag="dps")
                        nc.tensor.matmul(dps[:], tm_sb[:], bv_s[:],
                                         start=True, stop=True)
                        nc.scalar.activation(d_s[:, b:b + 1], dps[:],
                                             AF.Identity,
                                             bias=n1b_s[:, 0:1])

                if debug:
                    nc.sync.dma_start(dbg_xin[:], x_s[:])
                    nc.sync.dma_start(dbg_y[:], yhat[:])
                    for si in range(NS):
                        nc.sync.dma_start(dbg_acc[:, si, :],
                                          accflat(accr[si]))
                    nc.sync.dma_start(dbg_m2[:], M2T_s[:])

                # ---- phase 2a: x_mid = x + M2@yhat + d ---------------
                with tc.tile_pool(name="p2aps", bufs=3, space="PSUM") as aps:
                    for ch in range(B * NCH2):
                        b, rp = divmod(ch, NCH2)
                        yc = yhat[:, b, 2 * rp:2 * rp + 2, :]
                        xc = x_s[:, b, 2 * rp:2 * rp + 2, :]
                        pj = aps.tile([C, 2, W], f32, tag="pj")
                        nc.tensor.matmul(pj[:], M2T_s[:, b, :], yc,
                                         start=True, stop=False)
                        nc.tensor.matmul(pj[:], i128_s[:], xc,
                                         start=False, stop=True)
                        nc.scalar.activation(xc, pj[:], AF.Identity,
                                             bias=d_s[:, b:b + 1])
                if debug:
                    nc.sync.dma_start(dbg_xm[:], x_s[:])
            # yhat freed here

            # ==========================================================
            # Phase 2b: LN2 + folded FFN (fp8 DoubleRow), band by band
            # ==========================================================
            with tc.tile_pool(name="zp", bufs=1) as zp, \
                 tc.tile_pool(name="ln2", bufs=3) as lp2, \
                 tc.tile_pool(name="ln2ps", bufs=1, space="PSUM") as lps2, \
                 tc.tile_pool(name="fps", bufs=2, space="PSUM") as fps, \
                 tc.tile_pool(name="ops", bufs=2, space="PSUM") as ops_, \
                 tc.tile_pool(name="gp", bufs=2) as gp, \
                 tc.tile_pool(name="outp", bufs=3) as outp:

                zts = []
                for bd in range(NBAND):
                    zr0 = 16 * bd
                    zt = zp.tile([C, B, 18, WP], fp8, tag=f"zt{bd % 2}")
                    zts.append(zt)
                    for ch in range(2 * 9):
                        b, rp = divmod(ch, 9)
                        xc = x_s[:, b, zr0 + 2 * rp:zr0 + 2 * rp + 2, :]

                        def wout(dmu, rstd, zt=zt, b=b, rp=rp):
                            nc.vector.scalar_tensor_tensor(
                                zt[:, b, 2 * rp:2 * rp + 2, 1:W + 1],
                                dmu[:], n2w_s[:, 0:1], rstd[:],
                                OP.mult, OP.mult)
                        layernorm_chunk(xc, wout, lp2, lps2, bsplit=False)
                    if need_n2b:
                        nc.scalar.activation(zt[:, :, :, 1:W + 1],
                                             zt[:, :, :, 1:W + 1],
                                             AF.Identity,
                                             bias=n2b_s[:, 0:1])
                    # zero pad columns (left, right, scratch)
                    nc.vector.memset(zt[:, :, :, 0:1], 0.0)
                    nc.vector.memset(zt[:, :, :, W + 1:W + 3], 0.0)
                    # zero halo rows at image boundary
                    if bd == 0:
                        nc.vector.tensor_scalar(
                            zt[:, :, 0, 1:W + 1], zt[:, :, 0, 1:W + 1],
                            hm_s[:, 0:1], None, OP.mult)
                    if bd == NBAND - 1:
                        nc.vector.tensor_scalar(
                            zt[:, :, 17, 1:W + 1], zt[:, :, 17, 1:W + 1],
                            hm_s[:, 1:2], None, OP.mult)

                def dr_rhs(zt, b, rp, j):
                    """DoubleRow moving operand for tap-pair j of the
                    2-row chunk rp (batch b)."""
                    base = (zt.offset + b * 18 * WP + 2 * rp * WP)
                    pstride = zt.ap[0][0]
                    if j < 3:
                        return bass.AP(tensor=zt.tensor, offset=base + j,
                                       ap=[[pstride, C], [WP, 2],
                                           [WP, 2], [1, W]])
                    col = 0 if j == 3 else 2
                    return bass.AP(tensor=zt.tensor,
                                   offset=base + 2 * WP + col,
                                   ap=[[pstride, C], [1, 2],
                                       [WP, 2], [1, W]])

                def dr_lhs(p, h, j, mh):
                    off = (((p * 2 + h) * 5 + j) * 2) * C
                    return bass.AP(tensor=wf_s.tensor,
                                   offset=wf_s.offset + off,
                                   ap=[[wf_s.ap[0][0], C], [C, 2], [1, mh]])

                for bd in range(NBAND):
                    zr0 = 16 * bd
                    zt = zts[bd]
                    for ch in range(16):
                        b, rp = divmod(ch, 8)
                        ops = ops_.tile([C, 2, W], f32, tag="ops")
                        g01 = gp.tile([C, 2, 2, W], fp8, tag="g01")
                        g2 = gp.tile([C, 2, W], fp8, tag="g2")
                        for p in range(3):
                            mh = MH[p]
                            f1 = fps.tile([C, 2, W], f32, tag="f1")
                            f2 = fps.tile([C, 2, W], f32, tag="f2")
                            for j in range(5):
                                rhs = dr_rhs(zt, b, rp, j)
                                nc.tensor.matmul(
                                    f1[:mh], dr_lhs(p, 0, j, mh),
                                    rhs, start=(j == 0), stop=(j == 4),
                                    perf_mode=DR)
                                nc.tensor.matmul(
                                    f2[:mh], dr_lhs(p, 1, j, mh),
                                    rhs, start=(j == 0), stop=(j == 4),
                                    perf_mode=DR)
                            g1 = gp.tile([C, 2, W], f32, tag="g1")
                            nc.scalar.activation(g1[:mh], f1[:mh], AF.Gelu,
                                                 scale=1.0 / FS)
                            # g8 = 16 * gelu(f1) * f2  (fp8)
                            gdst = g01[:, p] if p < 2 else g2[:mh]
                            nc.vector.scalar_tensor_tensor(
                                gdst, g1[:mh], 16.0 / FS, f2[:mh],
                                OP.mult, OP.mult)
                        # out = (wpo*256) @ g8 = 4096 * out_true
                        nc.tensor.matmul(ops[:], wpo_s[:, 0:2, :], g01[:],
                                         start=True, stop=False,
                                         perf_mode=DR)
                        nc.tensor.matmul(ops[:], wpo_s[:MH[2], 2, :],
                                         g2[:MH[2]],
                                         start=False, stop=True)
                        o_sb = outp.tile([C, 2, W], f32, tag="o_sb")
                        nc.vector.scalar_tensor_tensor(
                            o_sb[:], ops[:], 1.0 / 4096.0,
                            x_s[:, b, zr0 + 1 + 2 * rp:zr0 + 3 + 2 * rp, :],
                            OP.mult, OP.add)
                        gr = 16 * bd + 2 * rp
                        nc.sync.dma_start(out_d[b, :, gr:gr + 2, :],
                                          o_sb[:])

    nc.compile()
    return nc


# ---------------------------------------------------------------------------
# host side
# ---------------------------------------------------------------------------

def _prep_inputs(inputs, nrows, n_cores):
    H = nrows * n_cores
    x = np.asarray(inputs["x"], np.float32)
    n1w = np.asarray(inputs["n1w"], np.float32)
    n1b = np.asarray(inputs["n1b"], np.float32)
    n2w = np.asarray(inputs["n2w"], np.float32)
    n2b = np.asarray(inputs["n2b"], np.float32)

    wqk_taps = np.zeros((NTAPS, C, 2 * CO), np.float32)
    bqk = np.zeros((2 * CO, NS), np.float32)
    ti = 0
    for si, r in enumerate(SCALES):
        wqk = np.asarray(inputs[f"wqk{si}"], np.float32)  # [64,128,r,r]
        wqkf = wqk * n1w[None, :, None, None]
        bqk[:, si] = np.einsum("ocyx,c->o", wqk, n1b)
        for dy in range(r):
            for dx in range(r):
                wqk_taps[ti] = wqkf[:, :, dy, dx].T
                ti += 1

    wv_cat = np.concatenate([np.asarray(inputs[f"wv{i}"], np.float32)[:, :, 0, 0]
                             for i in range(NS)], axis=0)      # [128,128]
    bv_cat = np.concatenate([np.asarray(inputs[f"bv{i}"], np.float32)
                             for i in range(NS)])
    bv_all = (wv_cat @ n1b + bv_cat).astype(np.float32)
    wv_f = (wv_cat * n1w[None, :])                             # [v, y]

    wpjT = np.asarray(inputs["wproj"], np.float32)[:, :, 0, 0].T.copy()

    wpi = np.asarray(inputs["wpi"], np.float32)[:, :, 0, 0]    # [680,128]
    wdw = np.asarray(inputs["wdw"], np.float32)[:, 0]          # [680,3,3]
    wf_full = np.zeros((9, C, 2 * HID), np.float32)
    for ti in range(9):
        dy, dx = divmod(ti, 3)
        wf_full[ti] = (wpi * wdw[:, dy, dx][:, None]).T        # [128,680]
    assert np.abs(wf_full).max() * FS < 230.0, "fp8 weight scale overflow"
    # DoubleRow pair table: (j, member) -> tap index (dy*3+dx)
    PAIRS = [(0, 3), (1, 4), (2, 5), (6, 7), (8, None)]
    wf8 = np.zeros((C, 3, 2, 5, 2, C), np.float32)
    for p in range(3):
        mh = min(C, HID - C * p)
        for h in range(2):
            c0 = h * HID + p * C
            for j, pair in enumerate(PAIRS):
                for i, tap in enumerate(pair):
                    if tap is None:
                        continue
                    wf8[:, p, h, j, i, :mh] = FS * wf_full[tap][:, c0:c0 + mh]
    wf8 = wf8.reshape(C, -1).astype(ml_dtypes.float8_e4m3)

    wpo = np.asarray(inputs["wpo"], np.float32)[:, :, 0, 0]    # [128,340]
    wpo_p = np.zeros((C, 3, C), np.float32)
    for p in range(3):
        mh = min(C, HID - C * p)
        wpo_p[:mh, p, :] = wpo[:, C * p:C * p + mh].T * 256.0
    assert np.abs(wpo_p).max() < 230.0, "fp8 wpo scale overflow"

    tv = np.array([[float(np.asarray(inputs[f"t{i}"]).reshape(-1)[0])
                    for i in range(NS)]], np.float32)
    eye2 = np.concatenate([np.eye(CO, dtype=np.float32)] * 2, axis=0)

    bf = ml_dtypes.bfloat16
    shared = {
        "wqk": wqk_taps.astype(bf), "bqk": bqk,
        "wvf": wv_f.astype(bf), "wpjT": wpjT.astype(bf),
        "dn1w": np.diag(n1w).astype(bf),
        "i128": np.eye(C, dtype=np.float32).astype(bf),
        "ones": np.ones((C, C), np.float32).astype(bf),
        "eye2": eye2, "eye64": np.eye(2 * CO, dtype=np.float32),
        "bv": bv_all.reshape(C, 1).astype(bf),
        "n1b": n1b.reshape(C, 1), "n2w": n2w.reshape(C, 1),
        "n2b": n2b.reshape(C, 1),
        "wf": wf8, "wpo": wpo_p.astype(ml_dtypes.float8_e4m3), "tvec": tv,
    }
    need_n2b = bool(np.any(n2b != 0.0))

    in_maps = []
    for i in range(n_cores):
        r0 = nrows * i
        slab = np.zeros((B, C, nrows + 2, W), np.float32)
        lo, hi = r0 - 1, r0 + nrows + 1
        slo, shi = max(lo, 0), min(hi, H)
        slab[:, :, slo - lo:shi - lo, :] = x[:, :, slo:shi, :]
        m = {"xs": slab.astype(bf),
             "hmask": np.array([[1.0 if i > 0 else 0.0,
                                 1.0 if i < n_cores - 1 else 0.0]],
                               np.float32)}
        m.update(shared)
        in_maps.append(m)
    return in_maps, need_n2b


def _run(nrows, n_cores, in_maps, need_n2b, trace=False):
    # A fresh uniquely-named program per call: re-executing an identical
    # NEFF on this stack yields corrupted results (device-persistent
    # state), so never reuse one.
    import uuid
    from concourse.bass_utils import run_bass_kernel_spmd
    nonce = uuid.uuid4().hex[:12]
    nc = _build(nrows, n_cores, need_n2b, nonce=nonce)
    for m in in_maps:
        m[f"nonce_{nonce}"] = np.zeros((1, 2), np.float32)
    return run_bass_kernel_spmd(nc, in_maps, core_ids=list(range(n_cores)),
                                trace=trace)


def run_sharded(inputs, nrows=32, n_cores=8, trace=False):
    in_maps, need_n2b = _prep_inputs(inputs, nrows, n_cores)
    res = _run(nrows, n_cores, in_maps, need_n2b, trace=trace)
    H = nrows * n_cores
    out = np.zeros((B, C, H, W), np.float32)
    for i in range(n_cores):
        out[:, :, nrows * i:nrows * (i + 1), :] = res.results[i]["out"]
    return out, res


def kernel(**inputs):
    out, _ = run_sharded(inputs, nrows=32, n_cores=8)
    return out
